# revision 4
# baseline (speedup 1.0000x reference)
"""Trainium2 Bass kernel for nn_MHSA_40346922778634.

Math (per batch b, head h; register group is computed-then-dropped by the
reference, so reg_qk/reg_v are dead inputs):
  X = x[b] as [C=512, N=1024]
  Q = Wq X + bq ; K = Wk X + bk ; V = Wv X + bv       (per head: [64, N])
  P_h = (rel_h + rel_w) reshaped [head, 64, N]
  E[i,j] = Q_h[:,i].K_h[:,j] + P_h[:,i].Q_h[:,j]      ([N, N])
  attn = softmax(E, axis=-1)
  Out_h = V_h @ attn^T ; out[b, h*64:(h+1)*64] = Out_h + X[h*64:(h+1)*64]

Kernel strategy (8 cores, data-parallel over batch, 2 batches/core):
  - E^T = Z^T U with U = [Q_h; P_h], Z = [K_h; Q_h] stacked to K=128
    (one matmul computes both energy terms), fp32 operands for accuracy.
  - exp without max-subtraction (logits bounded ~ +-35 -> safe in fp32),
    T^T = exp(E^T) stored bf16.
  - denominator via ones-augmented V^T (padded weight layout: 65 cols/head,
    65th col of V^T_pad is 1.0), AV matmuls in bf16.
  - normalize: reciprocal of row 64, broadcast via rank-1 matmul, DVE mul,
    residual add, store fp32.
"""

import sys

import numpy as np

try:
    import concourse.bass as bass  # noqa: F401
except Exception:  # pragma: no cover
    sys.path.insert(0, "/opt/trn_rl_repo")

import concourse.bass as bass  # noqa: F401
import concourse.tile as tile
from concourse import bacc, mybir
from concourse.bass_utils import run_bass_kernel_spmd

F32 = mybir.dt.float32
BF16 = mybir.dt.bfloat16
EXP = mybir.ActivationFunctionType.Exp

N_CORES = 8
B, C, WD, HD = 16, 512, 32, 32
HEAD, D, N = 8, 64, 1024
BPC = B // N_CORES  # batches per core


def build_bass():
    nc = bacc.Bacc("TRN2")

    xs_d = nc.dram_tensor("xs", [BPC, C, N], F32, kind="ExternalInput")
    wqt_d = nc.dram_tensor("wqt", [4, 128, 512], F32, kind="ExternalInput")
    wkt_d = nc.dram_tensor("wkt", [4, 128, 512], F32, kind="ExternalInput")
    wvpt_d = nc.dram_tensor("wvpt", [4, 128, 520], F32, kind="ExternalInput")
    bq_d = nc.dram_tensor("bq", [4, 128, 1], F32, kind="ExternalInput")
    bk_d = nc.dram_tensor("bk", [4, 128, 1], F32, kind="ExternalInput")
    bvp_d = nc.dram_tensor("bvp", [1, 520], F32, kind="ExternalInput")
    pos_d = nc.dram_tensor("pos", [HEAD, D, N], F32, kind="ExternalInput")
    out_d = nc.dram_tensor("out", [BPC, C, N], F32, kind="ExternalOutput")

    with tile.TileContext(nc) as tc:
        with (
            tc.tile_pool(name="consts", bufs=1) as cpool,
            tc.tile_pool(name="work", bufs=2) as wpool,
            tc.tile_pool(name="psum", bufs=4, space="PSUM") as pspool,
        ):
            # ---- constants ----
            wqt_sb = cpool.tile([128, 4, 512], F32, name="wqt_sb")
            wkt_sb = cpool.tile([128, 4, 512], F32, name="wkt_sb")
            wvpt_sb = cpool.tile([128, 4, 520], F32, name="wvpt_sb")
            for kc in range(4):
                nc.sync.dma_start(wqt_sb[:, kc, :], wqt_d[kc])
                nc.sync.dma_start(wkt_sb[:, kc, :], wkt_d[kc])
                nc.sync.dma_start(wvpt_sb[:, kc, :], wvpt_d[kc])
            bq_sb = cpool.tile([128, 4, 1], F32, name="bq_sb")
            bk_sb = cpool.tile([128, 4, 1], F32, name="bk_sb")
            for mc in range(4):
                nc.sync.dma_start(bq_sb[:, mc, :], bq_d[mc])
                nc.sync.dma_start(bk_sb[:, mc, :], bk_d[mc])
            bvp_sb = cpool.tile([1, 520], F32, name="bvp_sb")
            nc.sync.dma_start(bvp_sb[:], bvp_d[:])
            ones_sb = cpool.tile([128, 128], F32, name="ones_sb")
            nc.vector.memset(ones_sb[:], 1.0)
            zbias = cpool.tile([128, 1], F32, name="zbias")
            nc.vector.memset(zbias[:], 0.0)

            for b in range(BPC):
                # ---- load X ----
                x_sb = wpool.tile([128, 4, N], F32, name=f"x_{b}", tag="x")
                for kc in range(4):
                    nc.sync.dma_start(x_sb[:, kc, :], xs_d[b, kc * 128:(kc + 1) * 128, :])

                # ---- Q/K projections -> Qall/Kall (fp32, bias added) ----
                qall = wpool.tile([128, 4, N], F32, name=f"qall_{b}", tag="qall", bufs=1)
                kall = wpool.tile([128, 4, N], F32, name=f"kall_{b}", tag="kall", bufs=1)
                for (wt, bt, dst) in ((wqt_sb, bq_sb, qall), (wkt_sb, bk_sb, kall)):
                    for mc in range(4):
                        for nh in range(2):
                            ps = pspool.tile([128, 512], F32, name=f"ps_p{b}{mc}{nh}", tag="ps")
                            for kc in range(4):
                                nc.tensor.matmul(
                                    ps[:],
                                    wt[:, kc, mc * 128:(mc + 1) * 128],
                                    x_sb[:, kc, nh * 512:(nh + 1) * 512],
                                    start=(kc == 0),
                                    stop=(kc == 3),
                                )
                            nc.vector.tensor_scalar_add(
                                dst[:, mc, nh * 512:(nh + 1) * 512], ps[:], bt[:, mc, :]
                            )

                # ---- V^T padded projection (bf16 out) ----
                vpt = wpool.tile([128, 8, 520], BF16, name=f"vpt_{b}", tag="vpt")
                for nc8 in range(8):
                    ps = pspool.tile([128, 520], F32, name=f"ps_v{b}{nc8}", tag="ps")
                    for (lo, hi) in ((0, 512), (512, 520)):
                        for kc in range(4):
                            nc.tensor.matmul(
                                ps[:, lo:hi],
                                x_sb[:, kc, nc8 * 128:(nc8 + 1) * 128],
                                wvpt_sb[:, kc, lo:hi],
                                start=(kc == 0),
                                stop=False,
                            )
                        nc.tensor.matmul(
                            ps[:, lo:hi],
                            ones_sb[0:1, 0:128],
                            bvp_sb[:, lo:hi],
                            start=False,
                            stop=True,
                        )
                    nc.vector.tensor_copy(vpt[:, nc8, :], ps[:])

                # ---- per-head attention ----
                for h in range(8):
                    mc, r0 = h // 2, (h % 2) * 64
                    u_h = wpool.tile([128, N], F32, name=f"u_{b}_{h}", tag="u", bufs=3)
                    z_h = wpool.tile([128, N], F32, name=f"z_{b}_{h}", tag="z", bufs=3)
                    # U = [Q_h; P_h]
                    nc.sync.dma_start(u_h[64:128, :], pos_d[h])
                    if h % 2 == 0:
                        nc.vector.tensor_copy(u_h[0:64, :], qall[0:64, mc, :])
                        nc.vector.tensor_copy(z_h[0:64, :], kall[0:64, mc, :])
                        nc.sync.dma_start(z_h[64:128, :], qall[0:64, mc, :])
                    else:
                        nc.sync.dma_start(u_h[0:64, :], qall[64:128, mc, :])
                        nc.sync.dma_start(z_h[0:64, :], kall[64:128, mc, :])
                        nc.vector.tensor_copy(z_h[64:128, :], qall[64:128, mc, :])

                    # E^T chunks + exp
                    tts = []
                    for j in range(8):
                        eps = pspool.tile([128, N], F32, name=f"ps_e{b}{h}{j}", tag="ps")
                        for ih in range(2):
                            nc.tensor.matmul(
                                eps[:, ih * 512:(ih + 1) * 512],
                                z_h[:, j * 128:(j + 1) * 128],
                                u_h[:, ih * 512:(ih + 1) * 512],
                                start=True,
                                stop=True,
                            )
                        tt = wpool.tile([128, N], BF16, name=f"tt_{b}_{h}_{j}", tag="tt", bufs=10)
                        nc.scalar.activation(tt[:], eps[:], EXP, bias=zbias[:])
                        tts.append(tt)

                    # AV: O = V_aug @ T  (rows 0..63 numerator, row 64 denominator)
                    ops = pspool.tile([65, N], F32, name=f"ps_o{b}{h}", tag="ps")
                    for mh in range(2):
                        for j in range(8):
                            nc.tensor.matmul(
                                ops[:, mh * 512:(mh + 1) * 512],
                                vpt[:, j, h * 65:h * 65 + 65],
                                tts[j][:, mh * 512:(mh + 1) * 512],
                                start=(j == 0),
                                stop=(j == 7),
                            )

                    # normalize + residual
                    rre = wpool.tile([65, N], F32, name=f"r_{b}_{h}", tag="rre", bufs=1)
                    nc.vector.reciprocal(rre[64:65, :], ops[64:65, :])
                    rps = pspool.tile([64, N], F32, name=f"ps_r{b}{h}", tag="ps")
                    for mh in range(2):
                        nc.tensor.matmul(
                            rps[:, mh * 512:(mh + 1) * 512],
                            ones_sb[64:65, 0:64],
                            rre[64:65, mh * 512:(mh + 1) * 512],
                            start=True,
                            stop=True,
                        )
                    rb_sb = wpool.tile([64, N], F32, name=f"rb_{b}_{h}", tag="rb")
                    nc.vector.tensor_copy(rb_sb[:], rps[:])
                    osb = wpool.tile([64, N], F32, name=f"osb_{b}_{h}", tag="ostage", bufs=3)
                    nc.vector.tensor_mul(osb[:], ops[0:64, :], rb_sb[:])
                    xres = wpool.tile([64, N], F32, name=f"xres_{b}_{h}", tag="xres")
                    nc.sync.dma_start(xres[:], xs_d[b, h * 64:(h + 1) * 64, :])
                    fin = wpool.tile([64, N], F32, name=f"fin_{b}_{h}", tag="ostage", bufs=3)
                    nc.vector.tensor_add(fin[:], osb[:], xres[:])
                    nc.sync.dma_start(out_d[b, h * 64:(h + 1) * 64, :], fin[:])

    nc.compile()
    return nc


def _prep_consts(Wq, bq, Wk, bk, Wv, bv, rel_h, rel_w):
    wqt = np.ascontiguousarray(Wq.T).reshape(4, 128, 512).astype(np.float32)
    wkt = np.ascontiguousarray(Wk.T).reshape(4, 128, 512).astype(np.float32)
    wvpt = np.zeros((512, 520), np.float32)
    bvp = np.zeros((1, 520), np.float32)
    for h in range(HEAD):
        wvpt[:, h * 65:h * 65 + 64] = Wv[h * 64:(h + 1) * 64, :].T
        bvp[0, h * 65:h * 65 + 64] = bv[h * 64:(h + 1) * 64]
        bvp[0, h * 65 + 64] = 1.0
    pos = (rel_h + rel_w).reshape(HEAD, D, N).astype(np.float32)
    return {
        "wqt": wqt,
        "wkt": wkt,
        "wvpt": wvpt.reshape(4, 128, 520),
        "bq": bq.reshape(4, 128, 1).astype(np.float32),
        "bk": bk.reshape(4, 128, 1).astype(np.float32),
        "bvp": bvp,
        "pos": pos,
    }


_CACHE = {}


def kernel(x, Wq, bq, Wk, bk, Wv, bv, rel_h, rel_w, reg_qk, reg_v):
    # reg_qk / reg_v are computed-then-dropped by the reference -> unused.
    x = np.asarray(x, np.float32)
    consts = _prep_consts(
        *[np.asarray(a, np.float32) for a in (Wq, bq, Wk, bk, Wv, bv, rel_h, rel_w)]
    )
    xr = x.reshape(B, C, N)
    in_maps = []
    for c in range(N_CORES):
        m = dict(consts)
        m["xs"] = np.ascontiguousarray(xr[c * BPC:(c + 1) * BPC])
        in_maps.append(m)

    if "nc" not in _CACHE:
        _CACHE["nc"] = build_bass()
    res = run_bass_kernel_spmd(_CACHE["nc"], in_maps, list(range(N_CORES)))
    outs = [np.asarray(r["out"]) for r in res.results]
    return np.concatenate(outs, axis=0).reshape(B, C, WD, HD)


if __name__ == "__main__":
    nc = build_bass()
    print("built ok")
